# revision 25
# baseline (speedup 1.0000x reference)
"""QRNN forget-mult recurrence h_t = i_t*z_t + f_t*h_{t-1} on 8 NeuronCores.

Sharding: batch dim B=32 split 4-per-core (data parallel). Per core the
[T=4096, B=4, H=256] slice is marshaled on the host into channel-major
[C=1024, T] bf16 arrays, so the device kernel is a pure stream: DMA a
[128, T] tile per channel group, decode f, i*z on GpSimd, TensorTensorScan
on DVE (fp32 internal state regardless of operand dtype), DMA the bf16
result back out. bf16 I/O halves HBM traffic vs f32 — the binding roofline.

The forget gate travels as g = bf16(1 - f) and is decoded to f32 f = 1 - g
on the Activation engine: near f ~ 1 (where the recurrence error amplifies
as 1/(1-f)) the encoding keeps absolute precision ~(1-f)*2^-9 instead of
2^-9, cutting the worst-case output error ~3x at identical HBM traffic.
"""

import numpy as np
import ml_dtypes

BF16 = ml_dtypes.bfloat16

T = 4096
B = 32
H = 256
NCORES = 8
BS = B // NCORES          # batches per core
C = BS * H                # channels per core
P = 128                   # partitions
NG = C // P               # channel groups per core

_CACHE = {}


def _build_nc(SC=4096, ins_bufs=3, fd_bufs=2, iz_bufs=2, ht_bufs=2,
              mul_engine="gpsimd", scan_engine="vector", out_engine="gpsimd",
              in_engine="sync", repeat=1, skip=(), loop_order="sg",
              iz_dtype="f32", scan_src="f", in_mode="legacy"):
    import concourse.tile as tile
    from concourse import bacc, mybir

    f32 = mybir.dt.float32
    bf16 = mybir.dt.bfloat16
    mult = mybir.AluOpType.mult
    add = mybir.AluOpType.add
    copy_fn = mybir.ActivationFunctionType.Copy
    NS = T // SC

    nc = bacc.Bacc("TRN2", target_bir_lowering=False, debug=False)
    if in_mode == "packed":
        x_d = nc.dram_tensor("x", [C, 3, T], bf16, kind="ExternalInput")
    else:
        g_d = nc.dram_tensor("g", [C, T], bf16, kind="ExternalInput")
        i_d = nc.dram_tensor("i", [C, T], bf16, kind="ExternalInput")
        z_d = nc.dram_tensor("z", [C, T], bf16, kind="ExternalInput")
    h0_d = nc.dram_tensor("h0", [C, 1], f32, kind="ExternalInput")
    h_d = nc.dram_tensor("h", [C, T], bf16, kind="ExternalOutput")

    def eng_cycle(name, flip=0):
        if name == "split":
            engs = [nc.vector, nc.gpsimd]
            return lambda g: engs[(g + flip) % 2]
        e = getattr(nc, name)
        return lambda g: e

    mul_eng = eng_cycle(mul_engine, flip=1)
    scan_eng = eng_cycle(scan_engine)
    out_eng = getattr(nc, out_engine)
    if in_engine == "split":
        in_engs = [nc.sync, nc.scalar]
    else:
        in_engs = [getattr(nc, in_engine)]
    in_k = [0]

    def in_dma(dst, src):
        in_engs[in_k[0] % len(in_engs)].dma_start(dst, src)
        in_k[0] += 1

    with tile.TileContext(nc) as tc:
        with (
            tc.tile_pool(name="const", bufs=1) as constp,
            tc.tile_pool(name="ins", bufs=ins_bufs) as insp,
            tc.tile_pool(name="fdp", bufs=fd_bufs) as fdp,
            tc.tile_pool(name="izp", bufs=iz_bufs) as izp,
            tc.tile_pool(name="htp", bufs=ht_bufs) as htp,
        ):
            carries = []
            for g in range(NG):
                cg = constp.tile([P, 1], f32, tag=f"carry{g}")
                nc.sync.dma_start(cg[:], h0_d[g * P:(g + 1) * P, :])
                carries.append(cg)

            for r in range(repeat):
                prev = [None] * NG
                if loop_order == "sg":
                    sched = [(s, g) for s in range(NS) for g in range(NG)]
                else:
                    sched = [(s, g) for g in range(NG) for s in range(NS)]
                for s, g in sched:
                    ts = slice(s * SC, (s + 1) * SC)
                    if True:
                        gs = slice(g * P, (g + 1) * P)
                        gt = it = zt = None
                        if "in" not in skip and in_mode == "packed":
                            xt = insp.tile([P, 3, SC], bf16, tag="x")
                            in_dma(xt[:], x_d[gs, :, ts])
                            gt, it, zt = xt[:, 0], xt[:, 1], xt[:, 2]
                        elif "in" not in skip:
                            gtt = insp.tile([P, SC], bf16, tag="g")
                            itt = insp.tile([P, SC], bf16, tag="i")
                            ztt = insp.tile([P, SC], bf16, tag="z")
                            gt, it, zt = gtt[:], itt[:], ztt[:]
                            in_dma(gt, g_d[gs, ts])
                            in_dma(it, i_d[gs, ts])
                            in_dma(zt, z_d[gs, ts])

                        # f = 1 - g, decoded to f32 on the Activation engine
                        ft = None
                        if "dec" not in skip and scan_src == "f":
                            ft = fdp.tile([P, SC], f32, tag="f")
                            nc.scalar.activation(ft[:], gt, copy_fn,
                                                 bias=1.0, scale=-1.0)

                        izt = izp.tile(
                            [P, SC], f32 if iz_dtype == "f32" else bf16,
                            tag="iz")
                        if "mul" not in skip and it is not None:
                            mul_eng(g).tensor_mul(izt[:], it, zt)

                        ht = htp.tile([P, SC], bf16, tag="h")
                        init = (carries[g][:, 0:1] if s == 0
                                else prev[g][:, SC - 1:SC])
                        data0 = ft[:] if ft is not None else (
                            gt if gt is not None else izt[:])
                        if "scan" not in skip:
                            scan_eng(g).tensor_tensor_scan(
                                ht[:], data0, izt[:], init,
                                op0=mult, op1=add)
                        prev[g] = ht

                        if "out" not in skip:
                            out_eng.dma_start(h_d[gs, ts], ht[:])

    nc.compile()
    return nc


def _get_nc():
    if "nc" not in _CACHE:
        _CACHE["nc"] = _build_nc()
    return _CACHE["nc"]


def make_in_maps(f, z, i, hidden_init):
    f = np.asarray(f, dtype=np.float32)
    z = np.asarray(z, dtype=np.float32)
    i = np.asarray(i, dtype=np.float32)
    hidden_init = np.asarray(hidden_init, dtype=np.float32)
    one_minus_f = np.float32(1.0) - f
    in_maps = []
    for c in range(NCORES):
        b0 = c * BS
        # [T, BS, H] slice -> [C, T] channel-major bf16
        ct = lambda a: np.ascontiguousarray(
            a[:, b0:b0 + BS, :].reshape(T, C).T).astype(BF16)
        in_maps.append({
            "g": ct(one_minus_f),
            "i": ct(i),
            "z": ct(z),
            "h0": np.ascontiguousarray(
                hidden_init[b0:b0 + BS, :]).reshape(C, 1),
        })
    return in_maps


def kernel(f, z, i, hidden_init):
    import time

    from concourse.bass_utils import run_bass_kernel_spmd

    in_maps = make_in_maps(f, z, i, hidden_init)
    last_err = None
    for attempt in range(3):
        try:
            res = run_bass_kernel_spmd(
                _get_nc(), in_maps, list(range(NCORES))
            ).results
            break
        except Exception as e:  # transient device-unrecoverable states
            last_err = e
            time.sleep(2.0 * (attempt + 1))
    else:
        raise last_err
    out = np.empty((T, B, H), np.float32)
    for c in range(NCORES):
        hc = np.asarray(res[c]["h"]).reshape(BS, H, T)
        out[:, c * BS:(c + 1) * BS, :] = hc.transpose(2, 0, 1).astype(
            np.float32)
    return out


# revision 26
# speedup vs baseline: 1.4267x; 1.4267x over previous
"""QRNN forget-mult recurrence h_t = i_t*z_t + f_t*h_{t-1} on 8 NeuronCores.

Sharding: batch dim B=32 split 4-per-core (data parallel). Per core the
[T=4096, B=4, H=256] slice is marshaled on the host into channel-major
[C=1024, T] bf16 arrays, so the device kernel is a pure stream: DMA a
[128, T] tile per channel group, decode f, i*z on GpSimd, TensorTensorScan
on DVE (fp32 internal state regardless of operand dtype), DMA the bf16
result back out. bf16 I/O halves HBM traffic vs f32 — the binding roofline.

The forget gate travels as g = bf16(1 - f) and is decoded to f32 f = 1 - g
on the Activation engine: near f ~ 1 (where the recurrence error amplifies
as 1/(1-f)) the encoding keeps absolute precision ~(1-f)*2^-9 instead of
2^-9, cutting the worst-case output error ~3x at identical HBM traffic.
"""

import numpy as np
import ml_dtypes

BF16 = ml_dtypes.bfloat16

T = 4096
B = 32
H = 256
NCORES = 8
BS = B // NCORES          # batches per core
C = BS * H                # channels per core
P = 128                   # partitions
NG = C // P               # channel groups per core

_CACHE = {}


def _build_nc(SC=4096, ins_bufs=3, fd_bufs=2, iz_bufs=2, ht_bufs=2,
              mul_engine="vector", scan_engine="vector", out_engine="gpsimd",
              in_engine="sync", repeat=1, skip=(), loop_order="sg",
              iz_dtype="bf16", scan_src="f", in_mode="legacy"):
    import concourse.tile as tile
    from concourse import bacc, mybir

    f32 = mybir.dt.float32
    bf16 = mybir.dt.bfloat16
    mult = mybir.AluOpType.mult
    add = mybir.AluOpType.add
    copy_fn = mybir.ActivationFunctionType.Copy
    NS = T // SC

    nc = bacc.Bacc("TRN2", target_bir_lowering=False, debug=False)
    if in_mode == "packed":
        x_d = nc.dram_tensor("x", [C, 3, T], bf16, kind="ExternalInput")
    else:
        g_d = nc.dram_tensor("g", [C, T], bf16, kind="ExternalInput")
        i_d = nc.dram_tensor("i", [C, T], bf16, kind="ExternalInput")
        z_d = nc.dram_tensor("z", [C, T], bf16, kind="ExternalInput")
    h0_d = nc.dram_tensor("h0", [C, 1], f32, kind="ExternalInput")
    h_d = nc.dram_tensor("h", [C, T], bf16, kind="ExternalOutput")

    def eng_cycle(name, flip=0):
        if name == "split":
            engs = [nc.vector, nc.gpsimd]
            return lambda g: engs[(g + flip) % 2]
        e = getattr(nc, name)
        return lambda g: e

    mul_eng = eng_cycle(mul_engine, flip=1)
    scan_eng = eng_cycle(scan_engine)
    out_eng = getattr(nc, out_engine)
    if in_engine == "split":
        in_engs = [nc.sync, nc.scalar]
    else:
        in_engs = [getattr(nc, in_engine)]
    in_k = [0]

    def in_dma(dst, src):
        in_engs[in_k[0] % len(in_engs)].dma_start(dst, src)
        in_k[0] += 1

    with tile.TileContext(nc) as tc:
        with (
            tc.tile_pool(name="const", bufs=1) as constp,
            tc.tile_pool(name="ins", bufs=ins_bufs) as insp,
            tc.tile_pool(name="fdp", bufs=fd_bufs) as fdp,
            tc.tile_pool(name="izp", bufs=iz_bufs) as izp,
            tc.tile_pool(name="htp", bufs=ht_bufs) as htp,
        ):
            carries = []
            for g in range(NG):
                cg = constp.tile([P, 1], f32, tag=f"carry{g}")
                nc.sync.dma_start(cg[:], h0_d[g * P:(g + 1) * P, :])
                carries.append(cg)

            for r in range(repeat):
                prev = [None] * NG
                if loop_order == "sg":
                    sched = [(s, g) for s in range(NS) for g in range(NG)]
                else:
                    sched = [(s, g) for g in range(NG) for s in range(NS)]
                for s, g in sched:
                    ts = slice(s * SC, (s + 1) * SC)
                    if True:
                        gs = slice(g * P, (g + 1) * P)
                        gt = it = zt = None
                        if "in" not in skip and in_mode == "packed":
                            xt = insp.tile([P, 3, SC], bf16, tag="x")
                            in_dma(xt[:], x_d[gs, :, ts])
                            gt, it, zt = xt[:, 0], xt[:, 1], xt[:, 2]
                        elif "in" not in skip:
                            gtt = insp.tile([P, SC], bf16, tag="g")
                            itt = insp.tile([P, SC], bf16, tag="i")
                            ztt = insp.tile([P, SC], bf16, tag="z")
                            gt, it, zt = gtt[:], itt[:], ztt[:]
                            in_dma(gt, g_d[gs, ts])
                            in_dma(it, i_d[gs, ts])
                            in_dma(zt, z_d[gs, ts])

                        # f = 1 - g, decoded to f32 on the Activation engine
                        ft = None
                        if "dec" not in skip and scan_src == "f":
                            ft = fdp.tile([P, SC], f32, tag="f")
                            nc.scalar.activation(ft[:], gt, copy_fn,
                                                 bias=1.0, scale=-1.0)

                        izt = izp.tile(
                            [P, SC], f32 if iz_dtype == "f32" else bf16,
                            tag="iz")
                        if "mul" not in skip and it is not None:
                            mul_eng(g).tensor_mul(izt[:], it, zt)

                        ht = htp.tile([P, SC], bf16, tag="h")
                        init = (carries[g][:, 0:1] if s == 0
                                else prev[g][:, SC - 1:SC])
                        data0 = ft[:] if ft is not None else (
                            gt if gt is not None else izt[:])
                        if "scan" not in skip:
                            scan_eng(g).tensor_tensor_scan(
                                ht[:], data0, izt[:], init,
                                op0=mult, op1=add)
                        prev[g] = ht

                        if "out" not in skip:
                            out_eng.dma_start(h_d[gs, ts], ht[:])

    nc.compile()
    return nc


def _get_nc():
    if "nc" not in _CACHE:
        _CACHE["nc"] = _build_nc()
    return _CACHE["nc"]


def make_in_maps(f, z, i, hidden_init):
    f = np.asarray(f, dtype=np.float32)
    z = np.asarray(z, dtype=np.float32)
    i = np.asarray(i, dtype=np.float32)
    hidden_init = np.asarray(hidden_init, dtype=np.float32)
    one_minus_f = np.float32(1.0) - f
    in_maps = []
    for c in range(NCORES):
        b0 = c * BS
        # [T, BS, H] slice -> [C, T] channel-major bf16
        ct = lambda a: np.ascontiguousarray(
            a[:, b0:b0 + BS, :].reshape(T, C).T).astype(BF16)
        in_maps.append({
            "g": ct(one_minus_f),
            "i": ct(i),
            "z": ct(z),
            "h0": np.ascontiguousarray(
                hidden_init[b0:b0 + BS, :]).reshape(C, 1),
        })
    return in_maps


def kernel(f, z, i, hidden_init):
    import time

    from concourse.bass_utils import run_bass_kernel_spmd

    in_maps = make_in_maps(f, z, i, hidden_init)
    last_err = None
    for attempt in range(3):
        try:
            res = run_bass_kernel_spmd(
                _get_nc(), in_maps, list(range(NCORES))
            ).results
            break
        except Exception as e:  # transient device-unrecoverable states
            last_err = e
            time.sleep(2.0 * (attempt + 1))
    else:
        raise last_err
    out = np.empty((T, B, H), np.float32)
    for c in range(NCORES):
        hc = np.asarray(res[c]["h"]).reshape(BS, H, T)
        out[:, c * BS:(c + 1) * BS, :] = hc.transpose(2, 0, 1).astype(
            np.float32)
    return out
